# revision 1
# baseline (speedup 1.0000x reference)
"""DeepGCNLayer (GCNConv + GELU + LayerNorm) on 8 Trainium2 NeuronCores.

Strategy (pull-based, dst-sharded SPMD):
  - Reference math:  out = LN(gelu(segsum(norm * h[src]) + b)),  h = x @ W,
    norm = dinv[src] * dinv[dst], with self loops, deg over dst.
  - The dense matmul commutes with the segment sum, so we aggregate
    y[src] rows (y = dinv * x) first and run the small @W after:
        s[i]  = sum_{e: dst=i} y[src_e]   (self loop included)
        out_i = LN(gelu(dinv[i] * (s[i] @ W) + b))
  - Nodes (dst) are sharded across 8 cores; y is replicated so the heavy
    per-edge gather is core-local (no collectives).
  - The gather uses the GPSIMD dma_gather extended instruction (int16
    indices), so sources are split into 4 ranges of <=32767 rows; each
    128-edge block draws from one (dst-tile, range) group.  Blocks are
    scattered into dst columns via a one-hot is_equal matrix on the
    tensor engine, accumulating in PSUM per 128-node dst tile.
  - The SPMD program is shared across cores: per-(tile, range) block
    counts are the max over cores; shorter cores pad with a zero row.
"""

import numpy as np

# Problem constants (hardcoded per contract; kernel.py must be self-contained)
N = 100000
H = 128
NCORES = 8
P = 128
RANGE = 32767          # rows addressable by one int16 index range (pad row = 32767)
RANGE_STRIDE = 32768   # table stride per range (last row is the zero pad row)
NI = 2048              # indices per dma_gather call
BLK = NI // P          # 128-edge blocks per gather call


def _host_prep(x, edge_index):
    """Numpy preprocessing: sharding, degree norm, block schedule.

    Returns (shared_schedule, per_core_arrays, y_pad, dinv).
    """
    n, h = x.shape
    S = n // NCORES                      # dst nodes per core
    NT = (S + P - 1) // P                # dst tiles per core
    NR = (n + RANGE - 1) // RANGE        # source ranges

    src = np.asarray(edge_index[0]).astype(np.int64)
    dst = np.asarray(edge_index[1]).astype(np.int64)

    deg = np.bincount(dst, minlength=n).astype(np.float32) + 1.0
    dinv = (1.0 / np.sqrt(deg)).astype(np.float32)
    y = np.asarray(x, dtype=np.float32) * dinv[:, None]

    y_pad = np.zeros((NR * RANGE_STRIDE, h), np.float32)
    for r in range(NR):
        lo = r * RANGE
        hi = min(lo + RANGE, n)
        y_pad[r * RANGE_STRIDE: r * RANGE_STRIDE + (hi - lo)] = y[lo:hi]

    loop = np.arange(n, dtype=np.int64)
    src_all = np.concatenate([src, loop])
    dst_all = np.concatenate([dst, loop])

    core = dst_all // S
    per_core = []
    counts = np.zeros((NCORES, NT * NR), np.int64)
    for c in range(NCORES):
        m = core == c
        sc = src_all[m]
        dl = dst_all[m] - c * S
        t = dl // P
        r = sc // RANGE
        key = t * NR + r
        order = np.argsort(key, kind="stable")
        sc, dl, key = sc[order], dl[order], key[order]
        counts[c] = np.bincount(key, minlength=NT * NR)
        per_core.append((sc, dl, key))

    maxc = counts.max(axis=0)                        # [NT*NR]
    B = -(-maxc // P)                                # blocks per (t, r)
    B2 = B.reshape(NT, NR)

    # per-range block streams: G0[t, r] = block offset of group (t, r)
    # within range r's stream; L_r = total blocks of range r.
    G0 = np.zeros((NT, NR), np.int64)
    L_r = np.zeros(NR, np.int64)
    for r in range(NR):
        G0[:, r] = np.cumsum(B2[:, r]) - B2[:, r]
        L_r[r] = B2[:, r].sum()
    ncalls_r = [int(-(-L_r[r] // BLK)) if L_r[r] else 0 for r in range(NR)]
    call_base = np.cumsum([0] + ncalls_r)            # global call ids per range
    gcol_base = np.cumsum([0] + list(L_r))           # global dstloc col per range
    L_total = int(L_r.sum())
    ncalls_total = int(call_base[-1])

    # per-core device arrays; idx stored [P, ncalls * NI/16] for a clean DMA
    idx_all = np.full((NCORES, ncalls_total, P, NI // 16), RANGE, np.int16)
    dstloc = np.zeros((NCORES, P, max(L_total, 1)), np.float32)
    dinv_col = np.zeros((NCORES, P, NT), np.float32)

    grp_start = np.zeros(NT * NR + 1, np.int64)
    for c in range(NCORES):
        sc, dl, key = per_core[c]
        cnt = counts[c]
        grp_start[1:] = np.cumsum(cnt)
        # within-group offset for every edge
        offs = np.arange(len(key)) - grp_start[key]
        # flat position in range-r block stream (in edge slots)
        g0_flat = G0.reshape(-1)                     # index by key
        q = g0_flat[key] * P + offs                  # position within range stream
        r_of = key % NR
        t_of = key // NR
        dl_in_tile = (dl - t_of * P).astype(np.float32)

        for r in range(NR):
            mr = r_of == r
            if not mr.any():
                continue
            qr = q[mr]
            flat = np.full((ncalls_r[r] * NI,), RANGE, np.int16)
            flat[qr] = (sc[mr] % RANGE).astype(np.int16)
            # wrap: index i -> partition i%16, column i//16, replicated x8
            f2 = flat.reshape(ncalls_r[r], NI // 16, 16)
            idx_all[c, call_base[r]: call_base[r + 1], :, :] = np.tile(
                f2.transpose(0, 2, 1), (1, 8, 1)
            )
            # dstloc: column = global block, row = slot in block
            dcol = np.zeros((L_r[r] * P,), np.float32)
            dcol[qr] = dl_in_tile[mr]
            dstloc[c, :, gcol_base[r]: gcol_base[r + 1]] = (
                dcol.reshape(L_r[r], P).T
            )
        dv = np.zeros(NT * P, np.float32)
        dv[:S] = dinv[c * S: (c + 1) * S]
        dinv_col[c] = dv.reshape(NT, P).T

    sched = {
        "S": S, "NT": NT, "NR": NR,
        "B": B2, "G0": G0, "call_base": call_base, "gcol_base": gcol_base,
        "L_total": L_total, "ncalls_total": ncalls_total,
        "ncalls_r": ncalls_r,
    }
    # [C, ncalls, P, w] -> [C, P, ncalls*w]
    idx_flat = idx_all.transpose(0, 2, 1, 3).reshape(NCORES, P, -1).copy()
    arrays = {
        "idx_all": idx_flat, "dstloc": dstloc, "dinv_col": dinv_col,
    }
    return sched, arrays, y_pad


def _build_program(sched, h):
    import concourse.bacc as bacc
    import concourse.bass as bass
    import concourse.tile as tile
    from concourse import mybir

    S = sched["S"]
    NT = sched["NT"]
    NR = sched["NR"]
    B = sched["B"]
    G0 = sched["G0"]
    call_base = sched["call_base"]
    gcol_base = sched["gcol_base"]
    L_total = sched["L_total"]
    ncalls_total = sched["ncalls_total"]
    S_pad = NT * P

    nc = bacc.Bacc("TRN2", target_bir_lowering=False, debug=False,
                   enable_asserts=True, num_devices=NCORES)
    f32 = mybir.dt.float32

    ypad_d = nc.dram_tensor("ypad", [NR * RANGE_STRIDE, h], f32,
                            kind="ExternalInput").ap()
    idx_d = nc.dram_tensor("idx", [P, ncalls_total * (NI // 16)],
                           mybir.dt.int16, kind="ExternalInput").ap()
    dstloc_d = nc.dram_tensor("dstloc", [P, max(L_total, 1)], f32,
                              kind="ExternalInput").ap()
    dinv_d = nc.dram_tensor("dinvc", [P, NT], f32, kind="ExternalInput").ap()
    w_d = nc.dram_tensor("wmat", [h, h], f32, kind="ExternalInput").ap()
    b_d = nc.dram_tensor("bvec", [1, h], f32, kind="ExternalInput").ap()
    gam_d = nc.dram_tensor("gam", [1, h], f32, kind="ExternalInput").ap()
    bet_d = nc.dram_tensor("bet", [1, h], f32, kind="ExternalInput").ap()
    iota_d = nc.dram_tensor("iotar", [1, P], f32, kind="ExternalInput").ap()
    out_d = nc.dram_tensor("out", [S_pad, h], f32, kind="ExternalOutput").ap()

    def bcast(ap_row, parts=P):
        # DRAM [1, w] -> replicated [parts, w] access pattern
        return bass.AP(tensor=ap_row.tensor, offset=ap_row.offset,
                       ap=[[0, parts]] + ap_row.ap[1:])

    with tile.TileContext(nc) as tc:
        import contextlib
        with contextlib.ExitStack() as ctx:
            const = ctx.enter_context(tc.tile_pool(name="const", bufs=1))
            gpools = [
                ctx.enter_context(tc.tile_pool(name=f"gd{r}", bufs=3))
                for r in range(NR)
            ]
            spool = ctx.enter_context(tc.tile_pool(name="sel", bufs=3))
            stpool = ctx.enter_context(tc.tile_pool(name="st", bufs=3))
            epool = ctx.enter_context(tc.tile_pool(name="epi", bufs=3))
            ppool = ctx.enter_context(
                tc.tile_pool(name="pagg", bufs=2, space="PSUM"))
            opool = ctx.enter_context(
                tc.tile_pool(name="po", bufs=2, space="PSUM"))

            w_sb = const.tile([h, h], f32)
            nc.sync.dma_start(out=w_sb[:], in_=w_d[:, :])
            iota_sb = const.tile([P, P], f32)
            nc.gpsimd.dma_start(out=iota_sb[:], in_=bcast(iota_d[:, :]))
            b_sb = const.tile([P, h], f32)
            nc.gpsimd.dma_start(out=b_sb[:], in_=bcast(b_d[:, :]))
            gam_sb = const.tile([P, h], f32)
            nc.gpsimd.dma_start(out=gam_sb[:], in_=bcast(gam_d[:, :]))
            bet_sb = const.tile([P, h], f32)
            nc.gpsimd.dma_start(out=bet_sb[:], in_=bcast(bet_d[:, :]))
            eps_sb = const.tile([P, 1], f32)
            nc.vector.memset(eps_sb[:], 1e-5)
            dinv_sb = const.tile([P, NT], f32)
            nc.sync.dma_start(out=dinv_sb[:], in_=dinv_d[:, :])
            dstloc_sb = const.tile([P, max(L_total, 1)], f32)
            nc.sync.dma_start(out=dstloc_sb[:], in_=dstloc_d[:, :])
            idx_sb = const.tile([P, ncalls_total * (NI // 16)], mybir.dt.int16)
            nc.sync.dma_start(out=idx_sb[:], in_=idx_d[:, :])

            gdest = {}  # (r, call) -> tile

            def ensure_gather(r, call_local):
                key = (r, call_local)
                if key in gdest:
                    return gdest[key]
                dst_t = gpools[r].tile([P, BLK, h], f32, tag="gd")
                gcall = call_base[r] + call_local
                iw = NI // 16
                nc.gpsimd.dma_gather(
                    dst_t[:],
                    ypad_d[r * RANGE_STRIDE: (r + 1) * RANGE_STRIDE, :],
                    idx_sb[:, gcall * iw: (gcall + 1) * iw],
                    NI, NI, h,
                    single_packet=False,
                )
                gdest[key] = dst_t
                return dst_t

            for t in range(NT):
                blocks = []
                for r in range(NR):
                    for j in range(int(B[t, r])):
                        bp = int(G0[t, r]) + j
                        blocks.append((r, bp // BLK, bp % BLK,
                                       int(gcol_base[r]) + bp))
                psum_t = ppool.tile([h, P], f32)
                nb = len(blocks)
                for bi, (r, call_local, slot, gcol) in enumerate(blocks):
                    dst_t = ensure_gather(r, call_local)
                    sel = spool.tile([P, P], f32)
                    nc.vector.tensor_scalar(
                        out=sel[:], in0=iota_sb[:],
                        scalar1=dstloc_sb[:, gcol: gcol + 1], scalar2=None,
                        op0=mybir.AluOpType.is_equal,
                    )
                    nc.tensor.matmul(
                        out=psum_t[:],
                        lhsT=dst_t[:, slot, :],
                        rhs=sel[:],
                        start=(bi == 0), stop=(bi == nb - 1),
                    )
                # epilogue: sT -> SBUF, @W, dinv, +b, gelu, LN
                st_sb = stpool.tile([h, P], f32)
                nc.scalar.copy(out=st_sb[:], in_=psum_t[:])
                o_ps = opool.tile([P, h], f32)
                nc.tensor.matmul(out=o_ps[:], lhsT=st_sb[:], rhs=w_sb[:],
                                 start=True, stop=True)
                g = epool.tile([P, h], f32, tag="g")
                nc.vector.tensor_scalar(
                    out=g[:], in0=o_ps[:],
                    scalar1=dinv_sb[:, t: t + 1], scalar2=None,
                    op0=mybir.AluOpType.mult,
                )
                nc.vector.tensor_add(out=g[:], in0=g[:], in1=b_sb[:])
                nc.scalar.activation(out=g[:], in_=g[:],
                                     func=mybir.ActivationFunctionType.Gelu)
                stats = epool.tile([P, 6], f32, tag="stats")
                nc.vector.bn_stats(out=stats[:], in_=g[:])
                mv = epool.tile([P, 2], f32, tag="mv")
                nc.vector.bn_aggr(out=mv[:], in_=stats[:])
                rstd = epool.tile([P, 1], f32, tag="rstd")
                nc.scalar.activation(
                    out=rstd[:], in_=mv[:, 1:2],
                    func=mybir.ActivationFunctionType.Sqrt,
                    bias=eps_sb[:],
                )
                nc.vector.reciprocal(out=rstd[:], in_=rstd[:])
                nc.vector.tensor_scalar(
                    out=g[:], in0=g[:],
                    scalar1=mv[:, 0:1], scalar2=rstd[:],
                    op0=mybir.AluOpType.subtract,
                    op1=mybir.AluOpType.mult,
                )
                nc.vector.tensor_mul(out=g[:], in0=g[:], in1=gam_sb[:])
                nc.vector.tensor_add(out=g[:], in0=g[:], in1=bet_sb[:])
                nc.sync.dma_start(out=out_d[t * P: (t + 1) * P, :], in_=g[:])

    nc.compile()
    return nc


_last_results = None


def kernel(x, edge_index, W, b, gamma, beta):
    from concourse.bass_utils import run_bass_kernel_spmd

    x = np.asarray(x, np.float32)
    W = np.asarray(W, np.float32)
    b = np.asarray(b, np.float32)
    gamma = np.asarray(gamma, np.float32)
    beta = np.asarray(beta, np.float32)
    n, h = x.shape

    sched, arrays, y_pad = _host_prep(x, edge_index)
    nc = _build_program(sched, h)

    iota_row = np.arange(P, dtype=np.float32)[None, :]
    in_maps = []
    for c in range(NCORES):
        in_maps.append({
            "ypad": y_pad,
            "idx": arrays["idx_all"][c],
            "dstloc": arrays["dstloc"][c],
            "dinvc": arrays["dinv_col"][c],
            "wmat": W,
            "bvec": b[None, :],
            "gam": gamma[None, :],
            "bet": beta[None, :],
            "iotar": iota_row,
        })

    res = run_bass_kernel_spmd(nc, in_maps, core_ids=list(range(NCORES)))
    global _last_results
    _last_results = res
    S = sched["S"]
    out = np.concatenate(
        [res.results[c]["out"][:S] for c in range(NCORES)], axis=0
    )
    return out.astype(np.float32)



# revision 3
# speedup vs baseline: 1.4082x; 1.4082x over previous
"""DeepGCNLayer (GCNConv + GELU + LayerNorm) on 8 Trainium2 NeuronCores.

Strategy (pull-based, dst-sharded SPMD, chapter gather tables):
  - Reference math:  out = LN(gelu(segsum(norm * h[src]) + b)),  h = x @ W,
    norm = dinv[src] * dinv[dst], with self loops, deg over dst.
  - The dense matmul commutes with the segment sum:
        s[i]  = sum_{e: dst=i} y[src_e] + y[i],   y = dinv * x
        out_i = LN(gelu(dinv[i] * (s[i] @ W) + b))
  - dst tiles (128 nodes) are assigned to the 8 cores greedy-balanced by
    edge count; within a core, tiles are sorted by edge count so the
    shared SPMD schedule's per-slot max over cores is tight (~3% pad).
  - The per-edge gather uses GPSIMD dma_gather (int16 indices).  Each
    core's 98 tile slots are split into 7 "chapters" of 14 tiles; each
    chapter gets its own gather table holding the bf16 y rows of the
    chapter's unique sources (<32767 rows, so one int16 range).
  - Each 128-edge block is scatter-added into its dst tile's PSUM
    columns with a one-hot is_equal matrix on the tensor engine.
  - Self loops skip the gather: per tile, y_local rows are DMA-loaded
    sequentially and added via one matmul against the identity.
"""

import numpy as np
import ml_dtypes

# Problem constants (hardcoded per contract; kernel.py must be self-contained)
N = 100000
H = 128
NCORES = 8
P = 128
NT = 98            # dst tile slots per core
NT_G = NCORES * NT  # 784 global tile slots (782 real tiles + 2 phantom)
NCH = 7            # chapters per core
TPC = 14           # tile slots per chapter
CAP = 26624        # gather-table rows reserved per chapter (max unique + pad)
NI = 2048          # indices per dma_gather call
BLK = NI // P      # 16 blocks per gather call


def _host_prep(x, edge_index):
    n, h = x.shape
    src = np.asarray(edge_index[0]).astype(np.int64)
    dst = np.asarray(edge_index[1]).astype(np.int64)

    deg = np.bincount(dst, minlength=n).astype(np.float32) + 1.0
    dinv = (1.0 / np.sqrt(deg)).astype(np.float32)
    y16 = (np.asarray(x, np.float32) * dinv[:, None]).astype(ml_dtypes.bfloat16)

    # --- balanced tile -> (core, slot) assignment ---
    g_of = dst // P
    tile_cnt = np.bincount(g_of, minlength=NT_G)
    order = np.argsort(-tile_cnt, kind="stable")
    loads = np.zeros(NCORES, np.int64)
    counts = np.zeros(NCORES, np.int64)
    assign = [[] for _ in range(NCORES)]
    for g in order:
        elig = [c for c in range(NCORES) if counts[c] < NT]
        c = min(elig, key=lambda c: (loads[c], c))
        assign[c].append(g)
        loads[c] += tile_cnt[g]
        counts[c] += 1
    slot_tiles = np.zeros((NCORES, NT), np.int64)
    cnt = np.zeros((NCORES, NT), np.int64)
    for c in range(NCORES):
        ts = sorted(assign[c], key=lambda g: -tile_cnt[g])
        slot_tiles[c] = ts
        cnt[c] = tile_cnt[ts]

    # shared block schedule
    B = (-(-cnt // P)).max(axis=0)        # [NT] blocks per slot (ceil, max over cores)
    G0 = np.concatenate([[0], np.cumsum(B)])  # block prefix, G0[NT]=Ltot
    Ltot = int(G0[-1])
    CB = G0[np.arange(0, NT + 1, TPC)]    # chapter block bases [NCH+1]
    Lch = np.diff(CB)
    ncalls_ch = -(-Lch // BLK)
    call_base = np.concatenate([[0], np.cumsum(ncalls_ch)])
    ncalls = int(call_base[-1])

    # per-core maps for edges
    core_of_tile = np.zeros(NT_G, np.int64)
    slot_of_tile = np.zeros(NT_G, np.int64)
    for c in range(NCORES):
        core_of_tile[slot_tiles[c]] = c
        slot_of_tile[slot_tiles[c]] = np.arange(NT)
    e_core = core_of_tile[g_of]
    e_slot = slot_of_tile[g_of]

    ytab = np.zeros((NCORES, NCH * CAP, h), ml_dtypes.bfloat16)
    idx_all = np.zeros((NCORES, P, ncalls * (NI // 16)), np.int16)
    dstloc = np.zeros((NCORES, P, Ltot), np.float32)
    dinv_col = np.zeros((NCORES, P, NT), np.float32)
    ylocal = np.zeros((NCORES, NT * P, h), ml_dtypes.bfloat16)

    for c in range(NCORES):
        m = e_core == c
        sc = src[m]
        dl = dst[m] % P
        sl = e_slot[m]
        o = np.argsort(sl, kind="stable")
        sc, dl, sl = sc[o], dl[o], sl[o]
        slot_start = np.searchsorted(sl, np.arange(NT + 1))
        for ch in range(NCH):
            t0, t1 = ch * TPC, (ch + 1) * TPC
            e0, e1 = slot_start[t0], slot_start[t1]
            u = np.unique(sc[e0:e1])
            nu = len(u)
            assert nu + 1 <= CAP
            ytab[c, ch * CAP: ch * CAP + nu] = y16[u]
            # flat idx stream for this chapter (pad -> zero row at nu)
            flat = np.full(ncalls_ch[ch] * NI, nu, np.int16)
            dcol = np.zeros((Lch[ch] * P,), np.float32)
            for t in range(t0, t1):
                a, b = slot_start[t], slot_start[t + 1]
                q0 = (G0[t] - CB[ch]) * P
                q = q0 + np.arange(b - a)
                flat[q] = np.searchsorted(u, sc[a:b]).astype(np.int16)
                dcol[q] = dl[a:b]
            f2 = flat.reshape(ncalls_ch[ch], NI // 16, 16)
            idx_all[c, :, call_base[ch] * (NI // 16): call_base[ch + 1] * (NI // 16)] = (
                np.tile(f2.transpose(0, 2, 1), (1, 8, 1))
                .transpose(1, 0, 2).reshape(P, -1)
            )
            dstloc[c, :, CB[ch]: CB[ch + 1]] = dcol.reshape(Lch[ch], P).T
        # dinv / ylocal per slot
        for t in range(NT):
            g = slot_tiles[c, t]
            r0 = g * P
            r1 = min(r0 + P, n)
            k = max(0, r1 - r0)
            dv = np.ones(P, np.float32)
            if k > 0:
                dv[:k] = dinv[r0:r1]
                ylocal[c, t * P: t * P + k] = y16[r0:r1]
            dinv_col[c, :, t] = dv

    sched = {
        "B": B, "G0": G0, "CB": CB, "call_base": call_base,
        "Ltot": Ltot, "ncalls": ncalls, "slot_tiles": slot_tiles,
    }
    arrays = {
        "ytab": ytab, "idx": idx_all, "dstloc": dstloc,
        "dinvc": dinv_col, "ylocal": ylocal,
    }
    return sched, arrays


def _build_program(sched, h):
    import concourse.bacc as bacc
    import concourse.tile as tile
    from concourse import mybir

    B = sched["B"]
    G0 = sched["G0"]
    CB = sched["CB"]
    call_base = sched["call_base"]
    Ltot = sched["Ltot"]
    ncalls = sched["ncalls"]

    nc = bacc.Bacc("TRN2", target_bir_lowering=False, debug=False,
                   enable_asserts=True, num_devices=NCORES)
    f32 = mybir.dt.float32
    bf16 = mybir.dt.bfloat16

    ytab_d = nc.dram_tensor("ytab", [NCH * CAP, h], bf16, kind="ExternalInput").ap()
    idx_d = nc.dram_tensor("idx", [P, ncalls * (NI // 16)], mybir.dt.int16,
                           kind="ExternalInput").ap()
    dstloc_d = nc.dram_tensor("dstloc", [P, Ltot], f32, kind="ExternalInput").ap()
    dinv_d = nc.dram_tensor("dinvc", [P, NT], f32, kind="ExternalInput").ap()
    yloc_d = nc.dram_tensor("ylocal", [NT * P, h], bf16, kind="ExternalInput").ap()
    w_d = nc.dram_tensor("wmat", [h, h], f32, kind="ExternalInput").ap()
    b_d = nc.dram_tensor("bvec", [P, h], f32, kind="ExternalInput").ap()
    gam_d = nc.dram_tensor("gam", [P, h], f32, kind="ExternalInput").ap()
    bet_d = nc.dram_tensor("bet", [P, h], f32, kind="ExternalInput").ap()
    iota_d = nc.dram_tensor("iotar", [P, P], f32, kind="ExternalInput").ap()
    ident_d = nc.dram_tensor("ident", [P, P], bf16, kind="ExternalInput").ap()
    out_d = nc.dram_tensor("out", [NT * P, h], f32, kind="ExternalOutput").ap()

    with tile.TileContext(nc) as tc:
        import contextlib
        with contextlib.ExitStack() as ctx:
            const = ctx.enter_context(tc.tile_pool(name="const", bufs=1))
            gpool = ctx.enter_context(tc.tile_pool(name="gd", bufs=4))
            spool = ctx.enter_context(tc.tile_pool(name="sel", bufs=4))
            ylp = ctx.enter_context(tc.tile_pool(name="ylp", bufs=3))
            stpool = ctx.enter_context(tc.tile_pool(name="st", bufs=3))
            epool = ctx.enter_context(tc.tile_pool(name="epi", bufs=3))
            ppool = ctx.enter_context(
                tc.tile_pool(name="pagg", bufs=4, space="PSUM"))
            opool = ctx.enter_context(
                tc.tile_pool(name="po", bufs=4, space="PSUM"))

            w_sb = const.tile([h, h], f32)
            nc.sync.dma_start(out=w_sb[:], in_=w_d[:, :])
            iota_sb = const.tile([P, P], f32)
            nc.sync.dma_start(out=iota_sb[:], in_=iota_d[:, :])
            ident_sb = const.tile([P, P], bf16)
            nc.sync.dma_start(out=ident_sb[:], in_=ident_d[:, :])
            b_sb = const.tile([P, h], f32)
            nc.sync.dma_start(out=b_sb[:], in_=b_d[:, :])
            gam_sb = const.tile([P, h], f32)
            nc.sync.dma_start(out=gam_sb[:], in_=gam_d[:, :])
            bet_sb = const.tile([P, h], f32)
            nc.sync.dma_start(out=bet_sb[:], in_=bet_d[:, :])
            eps_sb = const.tile([P, 1], f32)
            nc.vector.memset(eps_sb[:], 1e-5)
            dinv_sb = const.tile([P, NT], f32)
            nc.sync.dma_start(out=dinv_sb[:], in_=dinv_d[:, :])
            dstloc_sb = const.tile([P, Ltot], f32)
            nc.sync.dma_start(out=dstloc_sb[:], in_=dstloc_d[:, :])
            idx_sb = const.tile([P, ncalls * (NI // 16)], mybir.dt.int16)
            nc.sync.dma_start(out=idx_sb[:], in_=idx_d[:, :])

            gdest = {}

            def ensure_gather(ch, call_local):
                key = (ch, call_local)
                if key in gdest:
                    return gdest[key]
                dst_t = gpool.tile([P, BLK, h], bf16, tag="gd")
                gcall = call_base[ch] + call_local
                iw = NI // 16
                nc.gpsimd.dma_gather(
                    dst_t[:],
                    ytab_d[ch * CAP: (ch + 1) * CAP, :],
                    idx_sb[:, gcall * iw: (gcall + 1) * iw],
                    NI, NI, h,
                    single_packet=False,
                )
                gdest[key] = dst_t
                return dst_t

            for ch in range(NCH):
                for t in range(ch * TPC, (ch + 1) * TPC):
                    psum_t = ppool.tile([h, P], f32)
                    nb = int(B[t])
                    for j in range(nb):
                        b = int(G0[t]) + j
                        lb = b - int(CB[ch])
                        dst_t = ensure_gather(ch, lb // BLK)
                        sel = spool.tile([P, P], bf16, tag="sel")
                        nc.vector.tensor_scalar(
                            out=sel[:], in0=iota_sb[:],
                            scalar1=dstloc_sb[:, b: b + 1], scalar2=None,
                            op0=mybir.AluOpType.is_equal,
                        )
                        nc.tensor.matmul(
                            out=psum_t[:],
                            lhsT=dst_t[:, lb % BLK, :],
                            rhs=sel[:],
                            start=(j == 0), stop=False,
                        )
                    yl = ylp.tile([P, h], bf16, tag="yl")
                    nc.sync.dma_start(out=yl[:], in_=yloc_d[t * P: (t + 1) * P, :])
                    nc.tensor.matmul(
                        out=psum_t[:], lhsT=yl[:], rhs=ident_sb[:],
                        start=False, stop=True,
                    )
                    # epilogue: sT -> SBUF, @W, dinv, +b, gelu, LN
                    st_sb = stpool.tile([h, P], f32)
                    nc.scalar.copy(out=st_sb[:], in_=psum_t[:])
                    o_ps = opool.tile([P, h], f32)
                    nc.tensor.matmul(out=o_ps[:], lhsT=st_sb[:], rhs=w_sb[:],
                                     start=True, stop=True)
                    g = epool.tile([P, h], f32, tag="g")
                    nc.vector.tensor_scalar(
                        out=g[:], in0=o_ps[:],
                        scalar1=dinv_sb[:, t: t + 1], scalar2=None,
                        op0=mybir.AluOpType.mult,
                    )
                    nc.vector.tensor_add(out=g[:], in0=g[:], in1=b_sb[:])
                    nc.scalar.activation(out=g[:], in_=g[:],
                                         func=mybir.ActivationFunctionType.Gelu)
                    stats = epool.tile([P, 6], f32, tag="stats")
                    nc.vector.bn_stats(out=stats[:], in_=g[:])
                    mv = epool.tile([P, 2], f32, tag="mv")
                    nc.vector.bn_aggr(out=mv[:], in_=stats[:])
                    rstd = epool.tile([P, 1], f32, tag="rstd")
                    nc.scalar.activation(
                        out=rstd[:], in_=mv[:, 1:2],
                        func=mybir.ActivationFunctionType.Sqrt,
                        bias=eps_sb[:],
                    )
                    nc.vector.reciprocal(out=rstd[:], in_=rstd[:])
                    nc.vector.tensor_scalar(
                        out=g[:], in0=g[:],
                        scalar1=mv[:, 0:1], scalar2=rstd[:],
                        op0=mybir.AluOpType.subtract,
                        op1=mybir.AluOpType.mult,
                    )
                    nc.vector.tensor_mul(out=g[:], in0=g[:], in1=gam_sb[:])
                    nc.vector.tensor_add(out=g[:], in0=g[:], in1=bet_sb[:])
                    nc.sync.dma_start(out=out_d[t * P: (t + 1) * P, :], in_=g[:])

    nc.compile()
    return nc


_last_results = None


def kernel(x, edge_index, W, b, gamma, beta):
    from concourse.bass_utils import run_bass_kernel_spmd

    x = np.asarray(x, np.float32)
    W = np.asarray(W, np.float32)
    b = np.asarray(b, np.float32)
    gamma = np.asarray(gamma, np.float32)
    beta = np.asarray(beta, np.float32)
    n, h = x.shape

    sched, arrays = _host_prep(x, edge_index)
    nc = _build_program(sched, h)

    iota = np.broadcast_to(np.arange(P, dtype=np.float32)[None, :], (P, P)).copy()
    ident = np.eye(P, dtype=ml_dtypes.bfloat16)
    in_maps = []
    for c in range(NCORES):
        in_maps.append({
            "ytab": arrays["ytab"][c],
            "idx": arrays["idx"][c],
            "dstloc": arrays["dstloc"][c],
            "dinvc": arrays["dinvc"][c],
            "ylocal": arrays["ylocal"][c],
            "wmat": W,
            "bvec": np.broadcast_to(b[None, :], (P, h)).copy(),
            "gam": np.broadcast_to(gamma[None, :], (P, h)).copy(),
            "bet": np.broadcast_to(beta[None, :], (P, h)).copy(),
            "iotar": iota,
            "ident": ident,
        })

    res = run_bass_kernel_spmd(nc, in_maps, core_ids=list(range(NCORES)))
    global _last_results
    _last_results = res

    slot_tiles = sched["slot_tiles"]
    out = np.zeros((n, h), np.float32)
    for c in range(NCORES):
        oc = res.results[c]["out"]
        for t in range(NT):
            g = int(slot_tiles[c, t])
            r0 = g * P
            if r0 >= n:
                continue
            r1 = min(r0 + P, n)
            out[r0:r1] = oc[t * P: t * P + (r1 - r0)]
    return out.astype(np.float32)


# revision 4
# speedup vs baseline: 1.4114x; 1.0023x over previous
"""DeepGCNLayer (GCNConv + GELU + LayerNorm) on 8 Trainium2 NeuronCores.

Strategy (pull-based, dst-sharded SPMD, chapter gather tables):
  - Reference math:  out = LN(gelu(segsum(norm * h[src]) + b)),  h = x @ W,
    norm = dinv[src] * dinv[dst], with self loops, deg over dst.
  - The dense matmul commutes with the segment sum:
        s[i]  = sum_{e: dst=i} y[src_e] + y[i],   y = dinv * x
        out_i = LN(gelu(dinv[i] * (s[i] @ W) + b))
  - dst tiles (128 nodes) are assigned to the 8 cores greedy-balanced by
    edge count; within a core, tiles are sorted by edge count so the
    shared SPMD schedule's per-slot max over cores is tight (~3% pad).
  - The per-edge gather uses GPSIMD dma_gather (int16 indices).  Each
    core's 98 tile slots are split into 7 "chapters" of 14 tiles; each
    chapter gets its own gather table holding the bf16 y rows of the
    chapter's unique sources (<32767 rows, so one int16 range).
  - Each 128-edge block is scatter-added into its dst tile's PSUM
    columns with a one-hot is_equal matrix on the tensor engine.
  - Self loops skip the gather: per tile, y_local rows are DMA-loaded
    sequentially and added via one matmul against the identity.
"""

import numpy as np
import ml_dtypes

# Problem constants (hardcoded per contract; kernel.py must be self-contained)
N = 100000
H = 128
NCORES = 8
P = 128
NT = 98            # dst tile slots per core
NT_G = NCORES * NT  # 784 global tile slots (782 real tiles + 2 phantom)
NCH = 7            # chapters per core
TPC = 14           # tile slots per chapter
CAP = 26624        # gather-table rows reserved per chapter (max unique + pad)
NI = 2048          # indices per dma_gather call
BLK = NI // P      # 16 blocks per gather call


def _host_prep(x, edge_index):
    n, h = x.shape
    src = np.asarray(edge_index[0]).astype(np.int64)
    dst = np.asarray(edge_index[1]).astype(np.int64)

    deg = np.bincount(dst, minlength=n).astype(np.float32) + 1.0
    dinv = (1.0 / np.sqrt(deg)).astype(np.float32)
    y16 = np.asarray(x, np.float32) * dinv[:, None]

    # --- balanced tile -> (core, slot) assignment ---
    g_of = dst // P
    tile_cnt = np.bincount(g_of, minlength=NT_G)
    order = np.argsort(-tile_cnt, kind="stable")
    loads = np.zeros(NCORES, np.int64)
    counts = np.zeros(NCORES, np.int64)
    assign = [[] for _ in range(NCORES)]
    for g in order:
        elig = [c for c in range(NCORES) if counts[c] < NT]
        c = min(elig, key=lambda c: (loads[c], c))
        assign[c].append(g)
        loads[c] += tile_cnt[g]
        counts[c] += 1
    slot_tiles = np.zeros((NCORES, NT), np.int64)
    cnt = np.zeros((NCORES, NT), np.int64)
    for c in range(NCORES):
        ts = sorted(assign[c], key=lambda g: -tile_cnt[g])
        slot_tiles[c] = ts
        cnt[c] = tile_cnt[ts]

    # shared block schedule
    B = (-(-cnt // P)).max(axis=0)        # [NT] blocks per slot (ceil, max over cores)
    G0 = np.concatenate([[0], np.cumsum(B)])  # block prefix, G0[NT]=Ltot
    Ltot = int(G0[-1])
    CB = G0[np.arange(0, NT + 1, TPC)]    # chapter block bases [NCH+1]
    Lch = np.diff(CB)
    ncalls_ch = -(-Lch // BLK)
    call_base = np.concatenate([[0], np.cumsum(ncalls_ch)])
    ncalls = int(call_base[-1])

    # per-core maps for edges
    core_of_tile = np.zeros(NT_G, np.int64)
    slot_of_tile = np.zeros(NT_G, np.int64)
    for c in range(NCORES):
        core_of_tile[slot_tiles[c]] = c
        slot_of_tile[slot_tiles[c]] = np.arange(NT)
    e_core = core_of_tile[g_of]
    e_slot = slot_of_tile[g_of]

    ytab = np.zeros((NCORES, NCH * CAP, h), np.float32)
    idx_all = np.zeros((NCORES, P, ncalls * (NI // 16)), np.int16)
    dstloc = np.zeros((NCORES, P, Ltot), np.float32)
    dinv_col = np.zeros((NCORES, P, NT), np.float32)
    ylocal = np.zeros((NCORES, NT * P, h), np.float32)

    for c in range(NCORES):
        m = e_core == c
        sc = src[m]
        dl = dst[m] % P
        sl = e_slot[m]
        o = np.argsort(sl, kind="stable")
        sc, dl, sl = sc[o], dl[o], sl[o]
        slot_start = np.searchsorted(sl, np.arange(NT + 1))
        for ch in range(NCH):
            t0, t1 = ch * TPC, (ch + 1) * TPC
            e0, e1 = slot_start[t0], slot_start[t1]
            u = np.unique(sc[e0:e1])
            nu = len(u)
            assert nu + 1 <= CAP
            ytab[c, ch * CAP: ch * CAP + nu] = y16[u]
            # flat idx stream for this chapter (pad -> zero row at nu)
            flat = np.full(ncalls_ch[ch] * NI, nu, np.int16)
            dcol = np.zeros((Lch[ch] * P,), np.float32)
            for t in range(t0, t1):
                a, b = slot_start[t], slot_start[t + 1]
                q0 = (G0[t] - CB[ch]) * P
                q = q0 + np.arange(b - a)
                flat[q] = np.searchsorted(u, sc[a:b]).astype(np.int16)
                dcol[q] = dl[a:b]
            f2 = flat.reshape(ncalls_ch[ch], NI // 16, 16)
            idx_all[c, :, call_base[ch] * (NI // 16): call_base[ch + 1] * (NI // 16)] = (
                np.tile(f2.transpose(0, 2, 1), (1, 8, 1))
                .transpose(1, 0, 2).reshape(P, -1)
            )
            dstloc[c, :, CB[ch]: CB[ch + 1]] = dcol.reshape(Lch[ch], P).T
        # dinv / ylocal per slot
        for t in range(NT):
            g = slot_tiles[c, t]
            r0 = g * P
            r1 = min(r0 + P, n)
            k = max(0, r1 - r0)
            dv = np.ones(P, np.float32)
            if k > 0:
                dv[:k] = dinv[r0:r1]
                ylocal[c, t * P: t * P + k] = y16[r0:r1]
            dinv_col[c, :, t] = dv

    sched = {
        "B": B, "G0": G0, "CB": CB, "call_base": call_base,
        "Ltot": Ltot, "ncalls": ncalls, "slot_tiles": slot_tiles,
    }
    arrays = {
        "ytab": ytab, "idx": idx_all, "dstloc": dstloc,
        "dinvc": dinv_col, "ylocal": ylocal,
    }
    return sched, arrays


def _build_program(sched, h):
    import concourse.bacc as bacc
    import concourse.tile as tile
    from concourse import mybir

    B = sched["B"]
    G0 = sched["G0"]
    CB = sched["CB"]
    call_base = sched["call_base"]
    Ltot = sched["Ltot"]
    ncalls = sched["ncalls"]

    nc = bacc.Bacc("TRN2", target_bir_lowering=False, debug=False,
                   enable_asserts=True, num_devices=NCORES)
    f32 = mybir.dt.float32
    bf16 = mybir.dt.bfloat16

    ytab_d = nc.dram_tensor("ytab", [NCH * CAP, h], f32, kind="ExternalInput").ap()
    idx_d = nc.dram_tensor("idx", [P, ncalls * (NI // 16)], mybir.dt.int16,
                           kind="ExternalInput").ap()
    dstloc_d = nc.dram_tensor("dstloc", [P, Ltot], f32, kind="ExternalInput").ap()
    dinv_d = nc.dram_tensor("dinvc", [P, NT], f32, kind="ExternalInput").ap()
    yloc_d = nc.dram_tensor("ylocal", [NT * P, h], f32, kind="ExternalInput").ap()
    w_d = nc.dram_tensor("wmat", [h, h], f32, kind="ExternalInput").ap()
    b_d = nc.dram_tensor("bvec", [P, h], f32, kind="ExternalInput").ap()
    gam_d = nc.dram_tensor("gam", [P, h], f32, kind="ExternalInput").ap()
    bet_d = nc.dram_tensor("bet", [P, h], f32, kind="ExternalInput").ap()
    iota_d = nc.dram_tensor("iotar", [P, P], f32, kind="ExternalInput").ap()
    ident_d = nc.dram_tensor("ident", [P, P], f32, kind="ExternalInput").ap()
    out_d = nc.dram_tensor("out", [NT * P, h], f32, kind="ExternalOutput").ap()

    with tile.TileContext(nc) as tc:
        import contextlib
        with contextlib.ExitStack() as ctx:
            const = ctx.enter_context(tc.tile_pool(name="const", bufs=1))
            gpool = ctx.enter_context(tc.tile_pool(name="gd", bufs=4))
            spool = ctx.enter_context(tc.tile_pool(name="sel", bufs=4))
            ylp = ctx.enter_context(tc.tile_pool(name="ylp", bufs=3))
            stpool = ctx.enter_context(tc.tile_pool(name="st", bufs=3))
            epool = ctx.enter_context(tc.tile_pool(name="epi", bufs=3))
            ppool = ctx.enter_context(
                tc.tile_pool(name="pagg", bufs=4, space="PSUM"))
            opool = ctx.enter_context(
                tc.tile_pool(name="po", bufs=4, space="PSUM"))

            w_sb = const.tile([h, h], f32)
            nc.sync.dma_start(out=w_sb[:], in_=w_d[:, :])
            iota_sb = const.tile([P, P], f32)
            nc.sync.dma_start(out=iota_sb[:], in_=iota_d[:, :])
            ident_sb = const.tile([P, P], f32)
            nc.sync.dma_start(out=ident_sb[:], in_=ident_d[:, :])
            b_sb = const.tile([P, h], f32)
            nc.sync.dma_start(out=b_sb[:], in_=b_d[:, :])
            gam_sb = const.tile([P, h], f32)
            nc.sync.dma_start(out=gam_sb[:], in_=gam_d[:, :])
            bet_sb = const.tile([P, h], f32)
            nc.sync.dma_start(out=bet_sb[:], in_=bet_d[:, :])
            eps_sb = const.tile([P, 1], f32)
            nc.vector.memset(eps_sb[:], 1e-5)
            dinv_sb = const.tile([P, NT], f32)
            nc.sync.dma_start(out=dinv_sb[:], in_=dinv_d[:, :])
            dstloc_sb = const.tile([P, Ltot], f32)
            nc.sync.dma_start(out=dstloc_sb[:], in_=dstloc_d[:, :])
            idx_sb = const.tile([P, ncalls * (NI // 16)], mybir.dt.int16)
            nc.sync.dma_start(out=idx_sb[:], in_=idx_d[:, :])

            gdest = {}

            def ensure_gather(ch, call_local):
                key = (ch, call_local)
                if key in gdest:
                    return gdest[key]
                dst_t = gpool.tile([P, BLK, h], f32, tag="gd")
                gcall = call_base[ch] + call_local
                iw = NI // 16
                nc.gpsimd.dma_gather(
                    dst_t[:],
                    ytab_d[ch * CAP: (ch + 1) * CAP, :],
                    idx_sb[:, gcall * iw: (gcall + 1) * iw],
                    NI, NI, h,
                    single_packet=False,
                )
                gdest[key] = dst_t
                return dst_t

            for ch in range(NCH):
                for t in range(ch * TPC, (ch + 1) * TPC):
                    psum_t = ppool.tile([h, P], f32)
                    nb = int(B[t])
                    for j in range(nb):
                        b = int(G0[t]) + j
                        lb = b - int(CB[ch])
                        dst_t = ensure_gather(ch, lb // BLK)
                        sel = spool.tile([P, P], f32, tag="sel")
                        nc.vector.tensor_scalar(
                            out=sel[:], in0=iota_sb[:],
                            scalar1=dstloc_sb[:, b: b + 1], scalar2=None,
                            op0=mybir.AluOpType.is_equal,
                        )
                        nc.tensor.matmul(
                            out=psum_t[:],
                            lhsT=dst_t[:, lb % BLK, :],
                            rhs=sel[:],
                            start=(j == 0), stop=False,
                        )
                    yl = ylp.tile([P, h], f32, tag="yl")
                    nc.sync.dma_start(out=yl[:], in_=yloc_d[t * P: (t + 1) * P, :])
                    nc.tensor.matmul(
                        out=psum_t[:], lhsT=yl[:], rhs=ident_sb[:],
                        start=False, stop=True,
                    )
                    # epilogue: sT -> SBUF, @W, dinv, +b, gelu, LN
                    st_sb = stpool.tile([h, P], f32)
                    nc.scalar.copy(out=st_sb[:], in_=psum_t[:])
                    o_ps = opool.tile([P, h], f32)
                    nc.tensor.matmul(out=o_ps[:], lhsT=st_sb[:], rhs=w_sb[:],
                                     start=True, stop=True)
                    g = epool.tile([P, h], f32, tag="g")
                    nc.vector.tensor_scalar(
                        out=g[:], in0=o_ps[:],
                        scalar1=dinv_sb[:, t: t + 1], scalar2=None,
                        op0=mybir.AluOpType.mult,
                    )
                    nc.vector.tensor_add(out=g[:], in0=g[:], in1=b_sb[:])
                    nc.scalar.activation(out=g[:], in_=g[:],
                                         func=mybir.ActivationFunctionType.Gelu)
                    stats = epool.tile([P, 6], f32, tag="stats")
                    nc.vector.bn_stats(out=stats[:], in_=g[:])
                    mv = epool.tile([P, 2], f32, tag="mv")
                    nc.vector.bn_aggr(out=mv[:], in_=stats[:])
                    rstd = epool.tile([P, 1], f32, tag="rstd")
                    nc.scalar.activation(
                        out=rstd[:], in_=mv[:, 1:2],
                        func=mybir.ActivationFunctionType.Sqrt,
                        bias=eps_sb[:],
                    )
                    nc.vector.reciprocal(out=rstd[:], in_=rstd[:])
                    nc.vector.tensor_scalar(
                        out=g[:], in0=g[:],
                        scalar1=mv[:, 0:1], scalar2=rstd[:],
                        op0=mybir.AluOpType.subtract,
                        op1=mybir.AluOpType.mult,
                    )
                    nc.vector.tensor_mul(out=g[:], in0=g[:], in1=gam_sb[:])
                    nc.vector.tensor_add(out=g[:], in0=g[:], in1=bet_sb[:])
                    nc.sync.dma_start(out=out_d[t * P: (t + 1) * P, :], in_=g[:])

    nc.compile()
    return nc


_last_results = None


def kernel(x, edge_index, W, b, gamma, beta):
    from concourse.bass_utils import run_bass_kernel_spmd

    x = np.asarray(x, np.float32)
    W = np.asarray(W, np.float32)
    b = np.asarray(b, np.float32)
    gamma = np.asarray(gamma, np.float32)
    beta = np.asarray(beta, np.float32)
    n, h = x.shape

    sched, arrays = _host_prep(x, edge_index)
    nc = _build_program(sched, h)

    iota = np.broadcast_to(np.arange(P, dtype=np.float32)[None, :], (P, P)).copy()
    ident = np.eye(P, dtype=np.float32)
    in_maps = []
    for c in range(NCORES):
        in_maps.append({
            "ytab": arrays["ytab"][c],
            "idx": arrays["idx"][c],
            "dstloc": arrays["dstloc"][c],
            "dinvc": arrays["dinvc"][c],
            "ylocal": arrays["ylocal"][c],
            "wmat": W,
            "bvec": np.broadcast_to(b[None, :], (P, h)).copy(),
            "gam": np.broadcast_to(gamma[None, :], (P, h)).copy(),
            "bet": np.broadcast_to(beta[None, :], (P, h)).copy(),
            "iotar": iota,
            "ident": ident,
        })

    res = run_bass_kernel_spmd(nc, in_maps, core_ids=list(range(NCORES)))
    global _last_results
    _last_results = res

    slot_tiles = sched["slot_tiles"]
    out = np.zeros((n, h), np.float32)
    for c in range(NCORES):
        oc = res.results[c]["out"]
        for t in range(NT):
            g = int(slot_tiles[c, t])
            r0 = g * P
            if r0 >= n:
                continue
            r1 = min(r0 + P, n)
            out[r0:r1] = oc[t * P: t * P + (r1 - r0)]
    return out.astype(np.float32)
